# revision 20
# baseline (speedup 1.0000x reference)
"""HQQ+SVD quantized linear for TRN2, tensor-parallel over out_features on 8 cores.

Math (approximates reference.py within ~9.5e-3 max-rel, gate is 2e-2):
  reference: W_f = (w-zp)*sc + up@dn;  out = (x_q @ Wq8.T)*sx*sw + bias
  kernel:    out = xqp @ W_f.T + bias  with xqp = fp16(x_q*sx)  (x-quant replicated
             exactly on host; the reference's W-requant noise dominates deviation)

Two per-o-tile decompositions, mixed to balance engines:

GROUP path (tiles 0..NT2-1, DVE-heavy):
  P_g[o,t]  = sum_{k in g} wT[k,o]*xqp[t,k]     (PE, raw int-valued fp8 weights,
                                                 PSUM layout [o, (g,t)])
  P_31     += CT[t,o] * 2^CSH                   (PE; one K=64 matmul: the whole
                                                 zp/bias/svd correction, rank<=T)
  Pc[o,t,g] = fp16(P)                           (ACT psum->sbuf copy, AP-transposed
                                                 to t-major so g is contiguous)
  S         = Pc * scT[o, g]                    (DVE fp16 TT 2x; compact scale
                                                 broadcast along t via stride-0 AP)
  out[o,t]  = sum_g S[o,t,g]                    (DVE pairwise add tree, tile-pair
                                                 fused ops)

2-STREAM path (last NS2 tiles, PE/DMA-heavy, zero DVE):
  v[o,k] = (sc[o,g]/s_o) * w[o,k] in [0,15], s_o = max_g sc[o,g]
  v ~= q1 + k2'  with q1 = floor(v+.5) in {0..15}, k2' = round((v-q1)*16)/16;
  both fp8e4m3-exact, stored as two fp8 streams (2 bytes/weight).  64 chained
  matmuls accumulate P[o,t] = sum_k (q1+k2')*xqp + CT2; ACT applies s_o via a
  single scale-copy.  |residual| <= 1/32 -> ~3.5e-3 extra rel err.  The last
  tile only carries its 96 real rows (M=96 matmuls).

All small constants ride one fat-lined "blob" DMA (tiny tensors with <2KB
partition lines crawl at descriptor rates).  gpsimd is left idle on purpose:
concurrent gpsimd work drops DVE to 1x.
"""
import sys
sys.path.insert(0, "/opt/trn_rl_repo")

import numpy as np
import concourse.bass as bass
import concourse.bacc as bacc
import concourse.tile as tile
import concourse.mybir as mybir

F32 = mybir.dt.float32
F16 = mybir.dt.float16
F8 = mybir.dt.float8e4
ALU = mybir.AluOpType

OUT, N_GROUPS, GROUP = 11008, 32, 128
IN = N_GROUPS * GROUP            # 4096
RANK = 128
T = 64
NCORES = 8
SHARD = OUT // NCORES            # 1376
PAD = 1408                       # 11 * 128
NTILES = PAD // 128              # 11
CSH = 6                          # global exponent shift for the C correction

NS2 = 4                          # trailing tiles on the 2-stream path
NT2 = NTILES - NS2               # group-path tiles
TREE_PAIRS = [(0, 1), (2, 3), (4, 5), (6,)]
assert sum(len(p) for p in TREE_PAIRS) == NT2

# per-tile M (output rows): last tile only has 96 real rows
TILE_M = [128] * (NTILES - 1) + [96]

# weight DRAM columns: group tiles IN cols; 2-stream tiles 2*NG*M cols
TILE_COL = []
_c = 0
for _i in range(NTILES):
    TILE_COL.append(_c)
    _c += IN if _i < NT2 else 2 * N_GROUPS * TILE_M[_i]
W_COLS = _c

# weight DMA chunks in column units; small starters first, 2-stream tiles
# split per stream so PE can start each tile after half its bytes
W_CHUNKS_COLS = [1024, 3072, 4096, 8192, 8192, 4096,
                 4096, 4096, 4096, 4096, 4096, 4096, 3072, 3072]
assert sum(W_CHUNKS_COLS) == W_COLS

# consts blob layout (fp16 columns): scT [128 rows] | ct [64 rows] | eye [64]
B_SCT, B_CT, B_EYE = 0, N_GROUPS * NTILES, N_GROUPS * NTILES + PAD
B_COLS = B_EYE + T

_nc_cache = {}


def _build():
    if "nc" in _nc_cache:
        return _nc_cache["nc"]
    nc = bacc.Bacc("TRN2", target_bir_lowering=False, debug=False)

    w_d = nc.dram_tensor("w", [128, W_COLS], F8, kind="ExternalInput")
    xqp_d = nc.dram_tensor("xqp", [128, N_GROUPS * T], F16, kind="ExternalInput")
    blob_d = nc.dram_tensor("blob", [128, B_COLS], F16, kind="ExternalInput")
    so_d = nc.dram_tensor("so", [128, NTILES], F32, kind="ExternalInput")
    out_d = nc.dram_tensor("out", [128, NTILES * T], F16, kind="ExternalOutput")

    XA = 8 * T                    # first xqp piece: groups 0..7

    with tile.TileContext(nc) as tc:
        with (
            tc.tile_pool(name="const", bufs=1) as cp,
            tc.tile_pool(name="work", bufs=4) as wp,
            tc.tile_pool(name="pp", bufs=4, space="PSUM") as pp,
            nc.allow_low_precision(reason="fp16 group-sum matches baseline"),
        ):
            # ---- starter pieces first: xqp groups 0-7, w tile-0 groups 0-7
            xqpA = cp.tile([128, XA], F16, tag="xqpA")
            nc.sync.dma_start(out=xqpA[:], in_=xqp_d[:, :XA])
            w_chunks = []
            col0 = 0
            for ci, ncols in enumerate(W_CHUNKS_COLS):
                # NB: distinct tag per chunk — same-tag tiles form one
                # rotating buffer set and serialize their DMAs behind use
                wt = cp.tile([128, ncols], F8, tag=f"wt{ci}")
                nc.sync.dma_start(out=wt[:], in_=w_d[:, col0:col0 + ncols])
                w_chunks.append((wt, col0))
                col0 += ncols
                if ci == 0:
                    # xqpB feeds tile-0 groups 8+: must precede chunk 1
                    xqpB = cp.tile([128, N_GROUPS * T - XA], tag="xqpB",
                                   dtype=F16)
                    nc.sync.dma_start(out=xqpB[:], in_=xqp_d[:, XA:])
                elif ci == 1:
                    # consts ride after tile-0's weights; blob (ct) is only
                    # needed at the very end of tile-0's matmul chain
                    blob_t = cp.tile([128, B_COLS], F16, tag="blob")
                    nc.sync.dma_start(out=blob_t[:], in_=blob_d[:])
                    so_t = cp.tile([128, NTILES], F32, tag="so")
                    nc.sync.dma_start(out=so_t[:], in_=so_d[:])

            scT_t = blob_t[:, B_SCT:B_SCT + N_GROUPS * NTILES]
            ct_t = blob_t[0:T, B_CT:B_CT + PAD]
            eye_t = blob_t[0:T, B_EYE:B_EYE + T]
            out_all = cp.tile([128, NTILES * T], F16, tag="outall")

            # ---- PE warm-up burst: dummy matmuls on a memset scratch tile
            # while the first weight DMAs are in flight.  The PE HAM clock
            # gate needs ~3.4us of activity to release 2.4 GHz; without this
            # the first ~3.4us of real matmuls run at 1.2 GHz.
            warm = cp.tile([128, 128], F16, tag="warm")
            nc.vector.memset(warm[:], 0)
            Pd = pp.tile([128, N_GROUPS * T // 2], F32, tag="p",
                         name="pdummy")
            for _ in range(64):
                nc.tensor.matmul(Pd[:, :T], warm[:], warm[:, :T],
                                 start=True, stop=True,
                                 skip_group_check=True)

            def xqp_slice(g):
                if g < 8:
                    return xqpA[:, g * T:(g + 1) * T]
                return xqpB[:, (g - 8) * T:(g - 7) * T]

            def w_slice(col, m):
                for wt, base in w_chunks:
                    if base <= col and col + m <= base + wt.shape[1]:
                        return wt[:, col - base:col - base + m]
                raise AssertionError("no chunk")

            def corr_mm(P_ap, i, m, stop):
                nc.tensor.matmul(P_ap, ct_t[:, i * 128:i * 128 + m], eye_t[:],
                                 start=False, stop=stop,
                                 skip_group_check=True)

            CT = N_GROUPS * T             # 2048 columns per tile
            # ---- group-path tiles, post-processing on ACT+DVE
            for pair in TREE_PAIRS:
                np_ = len(pair)
                S_t = wp.tile([128, np_ * CT], F16, tag="st",
                              name=f"st{pair[0]}")
                Pc_t = wp.tile([128, np_ * CT], F16, tag="pc",
                               name=f"pc{pair[0]}")
                for j, i in enumerate(pair):
                    # two psum halves per tile so extraction of half A
                    # overlaps PE work on half B (PSUM is only 8 banks)
                    HG = N_GROUPS // 2
                    for h in range(2):
                        Ph = pp.tile([128, HG * T], F32, tag="p")
                        for gg in range(HG):
                            g = h * HG + gg
                            qchain = g == N_GROUPS - 1
                            nc.tensor.matmul(Ph[:, gg * T:(gg + 1) * T],
                                             w_slice(TILE_COL[i] + g * 128,
                                                     128),
                                             xqp_slice(g),
                                             start=True, stop=not qchain,
                                             skip_group_check=True)
                            if qchain:
                                corr_mm(Ph[:, gg * T:(gg + 1) * T], i, 128,
                                        True)
                        # ACT copy with (g,t)->(t,g) transpose; half h fills
                        # g columns h*16..h*16+16 of the t-major Pc layout
                        P3 = Ph[:, :].rearrange("p (g t) -> p t g", g=HG)
                        Pc3 = Pc_t[:, j * CT:(j + 1) * CT].rearrange(
                            "p (t g) -> p t g",
                            g=N_GROUPS)[:, :, h * HG:(h + 1) * HG]
                        nc.scalar.copy(Pc3, P3)

                    S3 = S_t[:, j * CT:(j + 1) * CT].rearrange(
                        "p (t g) -> p t g", g=N_GROUPS)
                    sc3 = scT_t[:, i * N_GROUPS:(i + 1) * N_GROUPS
                                ].rearrange("p (o g) -> p o g", o=1)
                    _, sc3b = bass.broadcast_tensor_aps(S3, sc3)
                    Pc3f = Pc_t[:, j * CT:(j + 1) * CT].rearrange(
                        "p (t g) -> p t g", g=N_GROUPS)
                    nc.vector.tensor_tensor(S3, Pc3f, sc3b, ALU.mult)

                # pairwise add tree over contiguous g halves, both tiles of
                # the pair per op (jt = tile-major rows, 2x-packed on DVE)
                JT = np_ * T
                otile = out_all[:, pair[0] * T:pair[0] * T + JT]
                R_t = wp.tile([128, np_ * CT // 2], F16, tag="rt",
                              name=f"rt{pair[0]}")
                bufs = [S_t, R_t]
                w_half = N_GROUPS // 2
                src = S_t
                k = 0
                while w_half >= 1:
                    s3 = src[:, :JT * w_half * 2].rearrange(
                        "p (jt g) -> p jt g", g=w_half * 2)
                    if w_half == 1:
                        nc.vector.tensor_tensor(
                            otile,
                            s3[:, :, 0:1].rearrange("p jt g -> p (jt g)"),
                            s3[:, :, 1:2].rearrange("p jt g -> p (jt g)"),
                            ALU.add)
                    else:
                        dst = bufs[(k + 1) % 2]
                        d3 = dst[:, :JT * w_half].rearrange(
                            "p (jt g) -> p jt g", g=w_half)
                        nc.vector.tensor_tensor(d3, s3[:, :, :w_half],
                                                s3[:, :, w_half:], ALU.add)
                        src = dst
                    w_half //= 2
                    k += 1
                nc.sync.dma_start(
                    out=out_d[:, pair[0] * T:pair[0] * T + JT], in_=otile)

            # ---- 2-stream tiles: 64 chained matmuls + one ACT scale-copy
            for i in range(NT2, NTILES):
                m = TILE_M[i]
                P = pp.tile([128, N_GROUPS * T // 2], F32, tag="p")
                Pv = P[0:m, :T]
                for s in range(2):
                    for g in range(N_GROUPS):
                        nc.tensor.matmul(
                            Pv,
                            w_slice(TILE_COL[i] + (s * N_GROUPS + g) * m, m),
                            xqp_slice(g),
                            start=(s == 0 and g == 0), stop=False,
                            skip_group_check=True)
                corr_mm(Pv, i, m, True)
                nc.scalar.mul(out_all[0:m, i * T:(i + 1) * T], Pv,
                              so_t[0:m, i:i + 1])
            mid = (NT2 + 2) * T
            nc.sync.dma_start(out=out_d[:, NT2 * T:mid],
                              in_=out_all[:, NT2 * T:mid])
            nc.sync.dma_start(out=out_d[:, mid:], in_=out_all[:, mid:])

    nc.compile()
    _nc_cache["nc"] = nc
    return nc


def _prep_inputs(x, weight, scale, zero_point, svd_up, svd_down, bias):
    x = np.asarray(x, dtype=np.float32)
    weight = np.asarray(weight)
    scale = np.asarray(scale, dtype=np.float32)
    zero_point = np.asarray(zero_point, dtype=np.float32)
    svd_up = np.asarray(svd_up, dtype=np.float32)
    svd_down = np.asarray(svd_down, dtype=np.float32)
    bias = np.asarray(bias, dtype=np.float32)
    import ml_dtypes
    f8 = ml_dtypes.float8_e4m3

    # exact replication of reference's x-quant, then fold sx back in (fp16)
    xt = x.reshape(-1, IN)
    sx = (np.max(np.abs(xt), axis=1, keepdims=True) / np.float32(127.0))
    xq = np.clip(np.round(xt / sx), -128, 127).astype(np.float32)
    xqp = (xq * sx).astype(np.float16)                     # [T, IN]
    # xqp_d[p, g*T+t] = xqp[t, g*128+p]
    xqp_l = np.ascontiguousarray(
        xqp.T.reshape(N_GROUPS, 128, T).transpose(1, 0, 2).reshape(128, N_GROUPS * T))
    # sxg[t,g] = sum_{k in g} xqp[t,k], exact fp32 sum of the fp16 values
    sxg = xqp.astype(np.float32).reshape(T, N_GROUPS, 128).sum(axis=2)   # [T,32]
    # xd[r,t] = sum_k fp16(dn)[r,k] * xqp[t,k]
    xd = (svd_down.astype(np.float16).astype(np.float32)
          @ xqp.astype(np.float32).T)                         # [128, T] fp32
    eye = (np.eye(T, dtype=np.float32) * float(2 ** CSH))

    def pack_kgo(a):  # [m, NG, G] per tile -> [128p, (g, c)] columns
        m = a.shape[0]
        return a.transpose(2, 1, 0).reshape(128, N_GROUPS * m)

    npad = PAD - SHARD
    in_maps = []
    for c in range(NCORES):
        sl = slice(c * SHARD, (c + 1) * SHARD)
        w_c = np.concatenate([weight[sl].astype(np.float32),
                              np.zeros((npad, N_GROUPS, GROUP), np.float32)], 0)
        sc_c = np.concatenate([scale[sl], np.zeros((npad, N_GROUPS), np.float32)], 0)
        so_c = sc_c.max(axis=1)
        so_c[so_c == 0] = 1.0
        # weight streams per tile
        wl_parts = []
        for i in range(NTILES):
            tsl = slice(i * 128, i * 128 + TILE_M[i])
            if i < NT2:
                wl_parts.append(pack_kgo(w_c[tsl].astype(f8)))
            else:
                v = (sc_c[tsl] / so_c[tsl, None])[:, :, None] * w_c[tsl]
                q1 = np.floor(v + 0.5)
                k2 = np.round((v - q1) * 16.0) / 16.0
                wl_parts.append(pack_kgo(q1.astype(f8)))
                wl_parts.append(pack_kgo(k2.astype(f8)))
        w_l = np.ascontiguousarray(np.concatenate(wl_parts, axis=1))
        # scT[p, i*32+g] = sc[i*128+p, g]
        scT = np.ascontiguousarray(
            sc_c.reshape(NTILES, 128, N_GROUPS).transpose(1, 0, 2)
            .reshape(128, NTILES * N_GROUPS))
        so_l = np.ascontiguousarray(
            so_c.reshape(NTILES, 128).T).astype(np.float32)     # [128, NTILES]
        # full correction C[o,t] = -sum_g (zp*sc)[o,g]*sxg[t,g] + up@xd + bias;
        # group tiles ride P[g=31] (divide by sc31), 2-stream tiles by s_o;
        # both further divided by 2^CSH (eye carries it back)
        zp_c = np.concatenate([zero_point[sl],
                               np.zeros((npad, N_GROUPS), np.float32)], 0)
        bias_c = np.concatenate([bias[sl], np.zeros(npad, np.float32)])
        up_c = np.concatenate([svd_up[sl], np.zeros((npad, RANK), np.float32)], 0)
        C = (-(zp_c * sc_c) @ sxg.T + up_c @ xd + bias_c[:, None])  # [PAD, T]
        div = sc_c[:, N_GROUPS - 1].copy()
        div[div == 0] = 1.0
        div[NT2 * 128:] = so_c[NT2 * 128:]
        C = C / (div[:, None] * np.float32(2 ** CSH))
        assert np.abs(C).max() < 60000, f"C overflow {np.abs(C).max()}"
        blob = np.zeros((128, B_COLS), np.float32)
        blob[:, B_SCT:B_SCT + N_GROUPS * NTILES] = scT
        blob[0:T, B_CT:B_CT + PAD] = C.T
        blob[0:T, B_EYE:B_EYE + T] = eye
        in_maps.append(dict(w=w_l, xqp=xqp_l, so=so_l,
                            blob=blob.astype(np.float16)))
    return in_maps


def kernel(x, weight, scale, zero_point, svd_up, svd_down, bias):
    nc = _build()
    in_maps = _prep_inputs(x, weight, scale, zero_point, svd_up, svd_down, bias)
    _nc_cache["last_in_maps"] = in_maps
    from concourse.bass_utils import run_bass_kernel_spmd
    res = run_bass_kernel_spmd(nc, in_maps, core_ids=list(range(NCORES)))
    outs = [r["out"].astype(np.float32)
             .reshape(128, NTILES, T).transpose(1, 0, 2)
             .reshape(PAD, T)[:SHARD] for r in res.results]
    full = np.concatenate(outs, axis=0)                         # [OUT, T]
    return np.ascontiguousarray(full.T)[None].astype(np.float32)  # [1, T, OUT]
